# revision 1
# baseline (speedup 1.0000x reference)
"""Trainium2 Bass kernel for nn_MultiHeadAttention (B=4, S=2048, D=1024, H=16).

Sharding: 8 cores = 4 batches x 2 head-groups. Core c handles batch c//2,
heads [8*(c%2), 8*(c%2)+8). Each core computes qkv for its 8 heads,
attention, and a partial c_proj product using its 512 rows of W_proj.
Host sums the two partials per batch (the "all-reduce after c_proj").

Device-side layout choices (per core):
  - x arrives pre-transposed from host: xt = x[b].T  [D=1024, S=2048]
  - Q^T, K^T [128(=2 heads x 64), 4 pairs, S] fp32, V [128(s blk), 16, 512] bf16
  - scores^T per (pair, kb, qchunk) in PSUM, exp on ScalarE -> P^T bf16
  - attn^T accumulated in PSUM via V^T @ P^T (col-group packed head pairs)
  - softmax denominator: DVE bf16 accumulation of P^T over kb + ones-matmul
    partition reduce; no max subtraction (logits ~ N(0, 1/9), tiny)
  - c_proj: attn^T tiles feed matmul lhsT directly, partial out to DRAM
"""

import contextlib
import ctypes
import os
import sys
import types

import numpy as np

# ---------------------------------------------------------------------------
# NTFF profiling hook (used when BASS_PROBLEM_TRACE=1): the agent image lacks
# antenv.axon_hooks, so provide it via ctypes against libaxon_pjrt.so.
# ---------------------------------------------------------------------------
_AXON_SO = "/opt/axon/libaxon_pjrt.so"


def _install_ntff_hook():
    if "antenv.axon_hooks" in sys.modules:
        return
    try:
        import antenv
    except ImportError:
        return
    try:
        lib = ctypes.CDLL(_AXON_SO)
    except OSError:
        return
    if not hasattr(lib, "axon_start_nrt_profile"):
        return
    lib.axon_start_nrt_profile.argtypes = [
        ctypes.POINTER(ctypes.c_int64),
        ctypes.c_size_t,
    ]
    lib.axon_start_nrt_profile.restype = ctypes.c_int64
    lib.axon_stop_nrt_profile.argtypes = [ctypes.c_char_p]
    lib.axon_stop_nrt_profile.restype = ctypes.c_int64

    @contextlib.contextmanager
    def _hook(output_dir, device_ids):
        import jax

        jax.devices()
        if device_ids:
            ids = (ctypes.c_int64 * len(device_ids))(*device_ids)
            rc = lib.axon_start_nrt_profile(ids, len(device_ids))
        else:
            rc = lib.axon_start_nrt_profile(None, 0)
        if rc != 0:
            raise RuntimeError(f"axon_start_nrt_profile rc={rc}")
        try:
            yield
        finally:
            n = lib.axon_stop_nrt_profile(str(output_dir).encode())
            print(f"profile: {n} file(s) written to {output_dir}", file=sys.stderr)

    mod = types.ModuleType("antenv.axon_hooks")
    holder = [_hook]
    mod.get_axon_ntff_profile_hook = lambda: holder[0]
    mod.set_axon_ntff_profile_hook = lambda h: holder.__setitem__(0, h)
    sys.modules["antenv.axon_hooks"] = mod
    antenv.axon_hooks = mod


_install_ntff_hook()

# ---------------------------------------------------------------------------
# Problem constants (hardcoded per the contract)
# ---------------------------------------------------------------------------
B, S, D = 4, 2048, 1024
H, DK = 16, 64
N_CORES = 8
HPC = 8           # heads per core
NPAIR = HPC // 2  # head pairs per core = 4
FC = HPC * DK     # features per core = 512
SCALE = 1.0 / float(np.sqrt(DK))  # 0.125

_CACHED = {}


def _build():
    import concourse.tile as tile
    from concourse import bacc, mybir

    f32 = mybir.dt.float32
    f32r = mybir.dt.float32r
    bf16 = mybir.dt.bfloat16
    Exp = mybir.ActivationFunctionType.Exp

    nc = bacc.Bacc("TRN2", target_bir_lowering=False, debug=False,
                   num_devices=N_CORES)

    xt = nc.dram_tensor("xt", [D, S], f32r, kind="ExternalInput").ap()
    wq = nc.dram_tensor("wq", [D, FC], f32r, kind="ExternalInput").ap()
    wk = nc.dram_tensor("wk", [D, FC], f32r, kind="ExternalInput").ap()
    wv = nc.dram_tensor("wv", [D, FC], f32r, kind="ExternalInput").ap()
    wp = nc.dram_tensor("wp", [128, NPAIR, D], f32r, kind="ExternalInput").ap()
    out = nc.dram_tensor("out", [S, D], f32, kind="ExternalOutput").ap()

    KC = D // 128      # 8 contraction chunks for qkv
    SB = S // 128      # 16 seq blocks
    NQC = 2            # q chunks per seq
    QCW = S // NQC     # q chunk width = 1024
    KB = S // 128      # 16 key blocks

    with tile.TileContext(nc) as tc:
        with (
            tc.tile_pool(name="qkt", bufs=2) as qkt_pool,
            tc.tile_pool(name="vtl", bufs=1) as v_pool,
            tc.tile_pool(name="cst", bufs=1) as cst_pool,
            tc.tile_pool(name="xts", bufs=2) as xts_pool,
            tc.tile_pool(name="wqs", bufs=2) as wqs_pool,
        ):
            Vt = [v_pool.tile([128, 4, FC], bf16, tag=f"V{vc}",
                              name=f"V_{vc}") for vc in range(4)]
            ones = cst_pool.tile([128, 1], bf16, tag="ones")
            nc.gpsimd.memset(ones[:], 1.0)

            def load_xt_sc(sc):
                """Stream one 512-column slice of x^T: [128, KC, 512]."""
                t = xts_pool.tile([128, KC, 512], f32r, tag="XTs",
                                  name=f"xts_{sc}")
                for kc in range(KC):
                    nc.sync.dma_start(
                        t[:, kc, :],
                        xt[kc * 128:(kc + 1) * 128, sc * 512:(sc + 1) * 512])
                return t

            def load_w_pair(src_ap, p, nm):
                """One pair's [128, KC, 128] slice of wq/wk."""
                t = wqs_pool.tile([128, KC, 128], f32r, tag="Wslice",
                                  name=f"w_{nm}_{p}")
                for kc in range(KC):
                    nc.sync.dma_start(
                        t[:, kc, :],
                        src_ap[kc * 128:(kc + 1) * 128, p * 128:(p + 1) * 128])
                return t

            def emit_qk_pair_chunks(p, ps_pool, ps_bufs_tag):
                """Allocate QT/KT tiles for pair p; return (tiles, chunk
                emitters) — each chunk computes one 512-wide seq slice."""
                qtp = qkt_pool.tile([128, S], f32r, tag="QTp", name=f"qtp_{p}")
                ktp = qkt_pool.tile([128, S], f32r, tag="KTp", name=f"ktp_{p}")
                state = {}

                def emit_chunk(sc, p=p, xts=None):
                    if "wq" not in state:
                        state["wq"] = load_w_pair(wq, p, "q")
                        state["wk"] = load_w_pair(wk, p, "k")
                    if xts is None:
                        xts = load_xt_sc(sc)
                    ssl = slice(sc * 512, (sc + 1) * 512)
                    qps = ps_pool.tile([128, 512], f32, tag=ps_bufs_tag,
                                       name=f"qk_ps_{p}_{sc}_q")
                    for kc in range(KC):
                        nc.tensor.matmul(
                            qps[:], lhsT=state["wq"][:, kc, :],
                            rhs=xts[:, kc, :],
                            start=(kc == 0), stop=(kc == KC - 1))
                    nc.vector.tensor_copy(qtp[:, ssl], qps[:])
                    kps = ps_pool.tile([128, 512], f32, tag=ps_bufs_tag,
                                       name=f"qk_ps_{p}_{sc}_k")
                    for kc in range(KC):
                        nc.tensor.matmul(
                            kps[:], lhsT=state["wk"][:, kc, :],
                            rhs=xts[:, kc, :],
                            start=(kc == 0), stop=(kc == KC - 1))
                    nc.vector.tensor_copy(ktp[:, ssl], kps[:])

                return (qtp, ktp), emit_chunk

            # ---------------- prologue: V + pair-0 Q/K -------------------
            qk = {}
            with (
                tc.tile_pool(name="wvp", bufs=1) as wv_pool,
                tc.tile_pool(name="ps1", bufs=2, space="PSUM") as ps1,
            ):
                WV = wv_pool.tile([128, KC, FC], f32r, tag="WV")
                for kc in range(KC):
                    nc.sync.dma_start(WV[:, kc, :],
                                      wv[kc * 128:(kc + 1) * 128, :])
                qk[0], qk0_chunk = emit_qk_pair_chunks(0, ps1, "vps")
                for sc in range(4):
                    xts = load_xt_sc(sc)
                    qk0_chunk(sc, xts=xts)
                    for sbl in range(4):
                        vps = ps1.tile([128, FC], f32, tag="vps",
                                       name=f"vps_{sc}_{sbl}")
                        for kc in range(KC):
                            nc.tensor.matmul(
                                vps[:],
                                lhsT=xts[:, kc, sbl * 128:(sbl + 1) * 128],
                                rhs=WV[:, kc, :],
                                start=(kc == 0), stop=(kc == KC - 1))
                        nc.vector.tensor_copy(Vt[sc][:, sbl, :], vps[:])

            # ---------------- attention + interleaved qkv ----------------
            with (
                tc.tile_pool(name="atn", bufs=1) as attn_pool,
                tc.tile_pool(name="wpp", bufs=1) as wp_pool,
            ):
                ATN = attn_pool.tile([128, NPAIR, S], f32r, tag="ATN")
                WP = wp_pool.tile([128, NPAIR, D], f32r, tag="WP")
                for p in range(NPAIR):
                    nc.sync.dma_start(WP[:, p, :], wp[:, p, :])

                with (
                    tc.tile_pool(name="ptp", bufs=10) as pt_pool,
                    tc.tile_pool(name="dnp", bufs=2) as den_pool,
                    tc.tile_pool(name="dnq", bufs=1) as den2_pool,
                    tc.tile_pool(name="ivp", bufs=1) as inv_pool,
                    tc.tile_pool(name="stp", bufs=2, space="PSUM") as st_ps,
                    tc.tile_pool(name="avp", bufs=2, space="PSUM") as av_pool,
                    tc.tile_pool(name="dsp", bufs=1, space="PSUM") as dps_pool,
                    tc.tile_pool(name="qps2", bufs=1, space="PSUM") as qkv_ps,
                ):
                    def emit_boundary(pend):
                        """Close out a finished (p, qc) block: denominator
                        reduce, reciprocal, broadcast, divide -> ATN."""
                        (bp, bqc, bavs, baccA, baccB) = pend
                        bqoff = bqc * QCW
                        den = den2_pool.tile([128, 2, QCW], bf16, tag="den",
                                             name=f"den_{bp}_{bqc}")
                        nc.vector.tensor_add(den[:], baccA[:], baccB[:])
                        inv = inv_pool.tile([1, 2, QCW], f32, tag="inv",
                                            name=f"inv_{bp}_{bqc}")
                        for h in range(2):
                            for ns in range(2):
                                dps = dps_pool.tile([1, 512], f32, tag="dps",
                                                    name=f"dps_{bp}_{bqc}_{h}_{ns}")
                                nc.tensor.matmul(
                                    dps[:], lhsT=ones[:],
                                    rhs=den[:, h, ns * 512:(ns + 1) * 512],
                                    start=True, stop=True)
                                nc.vector.reciprocal_approx_fast(
                                    inv[0:1, h, ns * 512:(ns + 1) * 512],
                                    dps[:])
                        ibc = inv_pool.tile([128, 2, QCW], f32, tag="ibc",
                                            name=f"ibc_{bp}_{bqc}")
                        for h in range(2):
                            nc.gpsimd.partition_broadcast(
                                ibc[:, h, :], inv[0:1, h, :])
                        for h in range(2):
                            hsl = slice(64 * h, 64 * h + 64)
                            for ns in range(2):
                                nc.vector.tensor_mul(
                                    ATN[hsl, bp,
                                        bqoff + ns * 512:bqoff + (ns + 1) * 512],
                                    bavs[ns][hsl, :],
                                    ibc[hsl, h, ns * 512:(ns + 1) * 512])

                    pending = None
                    for p in range(NPAIR):
                        QTp, KTp = qk.pop(p)
                        for qc in range(NQC):
                            qoff = qc * QCW
                            avs = [av_pool.tile([128, 512], f32, tag="avps",
                                                name=f"avps_{p}_{qc}_{ns}")
                                   for ns in range(2)]
                            accA = den_pool.tile([128, 2, QCW], bf16,
                                                 tag="accA",
                                                 name=f"accA_{p}_{qc}")
                            accB = den_pool.tile([128, 2, QCW], bf16,
                                                 tag="accB",
                                                 name=f"accB_{p}_{qc}")
                            pts = {}

                            def sc_exp_den(kb, p=p, qc=qc, qoff=qoff,
                                           QTp=QTp, KTp=KTp,
                                           accA=accA, accB=accB, pts=pts):
                                for ns in range(2):
                                    nsl = slice(qoff + ns * 512,
                                                qoff + (ns + 1) * 512)
                                    st = st_ps.tile([128, 2, 512], f32,
                                                    tag="st",
                                                    name=f"st_{p}_{qc}_{kb}_{ns}")
                                    for h in range(2):
                                        hsl = slice(64 * h, 64 * h + 64)
                                        nc.tensor.matmul(
                                            st[:, h, :],
                                            lhsT=KTp[hsl,
                                                     kb * 128:(kb + 1) * 128],
                                            rhs=QTp[hsl, nsl],
                                            start=True, stop=True)
                                    pt = pt_pool.tile(
                                        [128, 2, 512], bf16, tag="pt",
                                        name=f"pt_{p}_{qc}_{kb}_{ns}")
                                    nc.scalar.activation(pt[:], st[:], Exp,
                                                         scale=SCALE)
                                    pts[(kb, ns)] = pt
                                    # denominator partials: A chain on DVE,
                                    # B chain on the otherwise-idle GpSimd
                                    acc = accA if kb % 2 == 0 else accB
                                    asl = acc[:, :, ns * 512:(ns + 1) * 512]
                                    if kb < 2:
                                        nc.vector.tensor_copy(asl, pt[:])
                                    else:
                                        nc.vector.tensor_add(asl, asl, pt[:])

                            def av(kb, p=p, avs=avs, pts=pts):
                                for ns in range(2):
                                    pt = pts.pop((kb, ns))
                                    for h in range(2):
                                        nc.tensor.matmul(
                                            avs[ns][64 * h:64 * h + 64, :],
                                            lhsT=Vt[kb // 4][
                                                :, kb % 4,
                                                p * 128 + 64 * h:
                                                p * 128 + 64 * h + 64],
                                            rhs=pt[:, h, :],
                                            start=(kb == 0),
                                            stop=(kb == KB - 1),
                                            tile_position=(0, 64 * h),
                                            skip_group_check=True)

                            # head start on scores/exp before closing out the
                            # previous block, so ScalarE never starves
                            filler = None
                            if qc == 0 and p + 1 < NPAIR:
                                qk[p + 1], filler = emit_qk_pair_chunks(
                                    p + 1, qkv_ps, "qk2")
                            sc_exp_den(0)
                            sc_exp_den(1)
                            sc_exp_den(2)
                            if pending is not None:
                                emit_boundary(pending)
                            av(0)
                            av(1)
                            av(2)
                            for kb in range(3, KB):
                                sc_exp_den(kb)
                                av(kb)
                                if filler is not None and kb in (3, 6, 9, 12):
                                    filler((kb - 3) // 3)
                            pending = (p, qc, avs, accA, accB)
                    emit_boundary(pending)

                # ---------------- c_proj partial -------------------------
                with (
                    tc.tile_pool(name="osb", bufs=3) as out_pool,
                    tc.tile_pool(name="pjp", bufs=2, space="PSUM") as pj_ps,
                ):
                    for sb in range(SB):
                        for nn in range(2):
                            pps = pj_ps.tile([128, 512], f32, tag="pps")
                            for p in range(NPAIR):
                                nc.tensor.matmul(
                                    pps[:],
                                    lhsT=ATN[:, p, sb * 128:(sb + 1) * 128],
                                    rhs=WP[:, p, nn * 512:(nn + 1) * 512],
                                    start=(p == 0), stop=(p == NPAIR - 1))
                            ot = out_pool.tile([128, 512], f32, tag="ot")
                            nc.vector.tensor_copy(ot[:], pps[:])
                            nc.sync.dma_start(
                                out[sb * 128:(sb + 1) * 128,
                                    nn * 512:(nn + 1) * 512],
                                ot[:])

    nc.compile()
    return nc


def _get_nc():
    if "nc" not in _CACHED:
        _CACHED["nc"] = _build()
    return _CACHED["nc"]


def _to_f32r(a):
    """Round fp32 to the fp32r grid (1s + 8e + 11m, low 12 bits zero, RNE)."""
    u = np.ascontiguousarray(a, dtype=np.float32).view(np.uint32).astype(np.uint64)
    u = (u + 0x7FF + ((u >> 12) & 1)) & 0xFFFFF000
    return u.astype(np.uint32).view(np.float32)


def _shard(x, W_attn, W_proj):
    """Build per-core input maps."""
    x = np.asarray(x, dtype=np.float32)
    W_attn = np.asarray(W_attn, dtype=np.float32)
    W_proj = np.asarray(W_proj, dtype=np.float32)
    in_maps = []
    for c in range(N_CORES):
        b, g = c // 2, c % 2
        fsl = slice(g * FC, (g + 1) * FC)
        in_maps.append({
            "xt": _to_f32r(x[b].T),
            "wq": _to_f32r(W_attn[:, 0 * D + g * FC:0 * D + (g + 1) * FC]),
            "wk": _to_f32r(W_attn[:, 1 * D + g * FC:1 * D + (g + 1) * FC]),
            "wv": _to_f32r(W_attn[:, 2 * D + g * FC:2 * D + (g + 1) * FC]),
            "wp": _to_f32r(
                W_proj[fsl, :].reshape(NPAIR, 128, D).transpose(1, 0, 2)),
        })
    return in_maps


def kernel(x, W_attn, W_proj):
    from concourse.bass_utils import run_bass_kernel_spmd

    nc = _get_nc()
    in_maps = _shard(x, W_attn, W_proj)
    trace = os.environ.get("BASS_PROBLEM_TRACE", "0") == "1"
    res = run_bass_kernel_spmd(nc, in_maps, list(range(N_CORES)), trace=trace)
    _CACHED["last_result"] = res
    out = np.empty((B, S, D), dtype=np.float32)
    for b in range(B):
        out[b] = res.results[2 * b]["out"] + res.results[2 * b + 1]["out"]
    return out



# revision 11
# speedup vs baseline: 1.1140x; 1.1140x over previous
"""Trainium2 Bass kernel for nn_MultiHeadAttention (B=4, S=2048, D=1024, H=16).

Sharding: 8 cores = 4 batches x 2 head-groups. Core c handles batch c//2,
heads [8*(c%2), 8*(c%2)+8). Host sums the two c_proj partials per batch.

v2 design (from trace analysis of the 503us baseline):
  - all matmul operands bf16 (FWL weight loads, half DMA bytes)
  - x^T resident in SBUF (loaded once, reused by all pairs' QK chains)
  - scores PSUM ring of 3 tiles [128,2h,512q] absorbs scheduling jitter so
    the ScalarE exp stream (the ~280us floor) never stalls
  - software-pipelined emission: scores lead AV by 4 steps so the PE queue
    never head-of-line blocks the exp stream
  - denominator: bf16 A/B accumulation chains on DVE; at block boundary a
    ones[128,64]-weighted matmul replicates sum-over-keys across all
    partitions (h0 -> psum rows 0:64, h1 -> 64:128, col-tiled), one DVE
    reciprocal + one tensor_mul normalizes avs -> ATN (no gpsimd bcast)
  - pair-outer / qc-inner(512) blocks; QK(p+1), V, and c_proj(qc) run as
    small PE filler quanta inside the block streams; c_proj overlaps the
    pair-3 blocks so the tail is one q-chunk
"""

import contextlib
import ctypes
import os
import sys
import types

import numpy as np
import ml_dtypes

# ---------------------------------------------------------------------------
# NTFF profiling hook (used when BASS_PROBLEM_TRACE=1)
# ---------------------------------------------------------------------------
_AXON_SO = "/opt/axon/libaxon_pjrt.so"


def _install_ntff_hook():
    if "antenv.axon_hooks" in sys.modules:
        return
    try:
        import antenv
    except ImportError:
        return
    try:
        lib = ctypes.CDLL(_AXON_SO)
    except OSError:
        return
    if not hasattr(lib, "axon_start_nrt_profile"):
        return
    lib.axon_start_nrt_profile.argtypes = [
        ctypes.POINTER(ctypes.c_int64),
        ctypes.c_size_t,
    ]
    lib.axon_start_nrt_profile.restype = ctypes.c_int64
    lib.axon_stop_nrt_profile.argtypes = [ctypes.c_char_p]
    lib.axon_stop_nrt_profile.restype = ctypes.c_int64

    @contextlib.contextmanager
    def _hook(output_dir, device_ids):
        import jax

        jax.devices()
        if device_ids:
            ids = (ctypes.c_int64 * len(device_ids))(*device_ids)
            rc = lib.axon_start_nrt_profile(ids, len(device_ids))
        else:
            rc = lib.axon_start_nrt_profile(None, 0)
        if rc != 0:
            raise RuntimeError(f"axon_start_nrt_profile rc={rc}")
        try:
            yield
        finally:
            n = lib.axon_stop_nrt_profile(str(output_dir).encode())
            print(f"profile: {n} file(s) written to {output_dir}", file=sys.stderr)

    mod = types.ModuleType("antenv.axon_hooks")
    holder = [_hook]
    mod.get_axon_ntff_profile_hook = lambda: holder[0]
    mod.set_axon_ntff_profile_hook = lambda h: holder.__setitem__(0, h)
    sys.modules["antenv.axon_hooks"] = mod
    antenv.axon_hooks = mod


_install_ntff_hook()

# ---------------------------------------------------------------------------
# Problem constants (hardcoded per the contract)
# ---------------------------------------------------------------------------
B, S, D = 4, 2048, 1024
H, DK = 16, 64
N_CORES = 8
HPC = 8            # heads per core
NPAIR = HPC // 2   # head pairs per core = 4
FC = HPC * DK      # features per core = 512
SCALE = 1.0 / float(np.sqrt(DK))  # 0.125

KC = D // 128      # 8 contraction chunks for qkv projections
NSC = 4            # seq chunks of 512 for x / QK tiles
KB = S // 128      # 16 key blocks
NQC = 4            # q chunks of 512
LAG = 4            # AV trails scores by this many kb steps

_CACHED = {}


def _build():
    import concourse.tile as tile
    from concourse import bacc, mybir

    f32 = mybir.dt.float32
    bf16 = mybir.dt.bfloat16
    Exp = mybir.ActivationFunctionType.Exp

    nc = bacc.Bacc("TRN2", target_bir_lowering=False, debug=False,
                   num_devices=N_CORES)

    # Pre-swizzled DRAM inputs (host packs these; contiguous per partition)
    xs = nc.dram_tensor("xs", [NSC, 128, KC, 512], bf16,
                        kind="ExternalInput").ap()
    wqk = nc.dram_tensor("wqk", [NPAIR, 128, 2, KC, 128], bf16,
                         kind="ExternalInput").ap()
    wv = nc.dram_tensor("wv", [128, KC, FC], bf16, kind="ExternalInput").ap()
    wp = nc.dram_tensor("wp", [128, NPAIR, D], bf16,
                        kind="ExternalInput").ap()
    out = nc.dram_tensor("out", [S, D], f32, kind="ExternalOutput").ap()
    dbg_on = os.environ.get("BASS_DEBUG_DUMP", "0") == "1"
    if dbg_on:
        dqt = nc.dram_tensor("dqt", [128, S], bf16, kind="ExternalOutput").ap()
        dkt = nc.dram_tensor("dkt", [128, S], bf16, kind="ExternalOutput").ap()
        dvt = nc.dram_tensor("dvt", [128, KB, FC], bf16,
                             kind="ExternalOutput").ap()
        datn = nc.dram_tensor("datn", [128, NPAIR, S], bf16,
                              kind="ExternalOutput").ap()
        dacc = nc.dram_tensor("dacc", [128, 2, 512], bf16,
                              kind="ExternalOutput").ap()

    with tile.TileContext(nc) as tc:
        with (
            tc.tile_pool(name="xsp", bufs=NSC) as xs_pool,
            tc.tile_pool(name="wqkp", bufs=NPAIR) as wqk_pool,
            tc.tile_pool(name="wvp", bufs=1) as wv_pool,
            tc.tile_pool(name="wpp", bufs=1) as wp_pool,
            tc.tile_pool(name="qkp", bufs=2) as qk_pool,
            tc.tile_pool(name="vtp", bufs=1) as v_pool,
            tc.tile_pool(name="ptp", bufs=8) as pt_pool,
            tc.tile_pool(name="accp", bufs=2) as acc_pool,
            tc.tile_pool(name="invp", bufs=2) as inv_pool,
            tc.tile_pool(name="atnp", bufs=1) as atn_pool,
            tc.tile_pool(name="outp", bufs=3) as out_pool,
            tc.tile_pool(name="cstp", bufs=1) as cst_pool,
            tc.tile_pool(name="stp", bufs=3, space="PSUM") as st_pool,
            tc.tile_pool(name="avp", bufs=1, space="PSUM") as av_pool,
            tc.tile_pool(name="shp", bufs=1, space="PSUM") as sh_pool,
        ):
            # ---------------- static tiles ----------------
            ones = cst_pool.tile([128, 64], bf16, tag="ones")
            nc.gpsimd.memset(ones[:], 1.0)

            XS = [xs_pool.tile([128, KC, 512], bf16, tag="xs",
                               name=f"xs_{sc}") for sc in range(NSC)]
            WV = wv_pool.tile([128, KC, FC], bf16, tag="wv")
            WP = wp_pool.tile([128, NPAIR, D], bf16, tag="wp")
            Vt = v_pool.tile([128, KB, FC], bf16, tag="vt")
            ATN = atn_pool.tile([128, NPAIR, S], bf16, tag="atn")

            WQK = []
            for p in range(NPAIR):
                t = wqk_pool.tile([128, 2, KC, 128], bf16, tag="wqk",
                                  name=f"wqk_{p}")
                WQK.append(t)

            # DMA order: pair0 weights + x chunk 0 first (prologue critical
            # path), then everything else.
            nc.sync.dma_start(WQK[0][:], wqk[0])
            nc.sync.dma_start(XS[0][:], xs[0])
            nc.sync.dma_start(WV[:], wv[:])
            for sc in range(1, NSC):
                nc.sync.dma_start(XS[sc][:], xs[sc])
            for p in range(1, NPAIR):
                nc.sync.dma_start(WQK[p][:], wqk[p])
            nc.sync.dma_start(WP[:], wp[:])

            # QT/KT tiles per pair (ring of 2)
            QK = {}

            def alloc_qk(p):
                qt = qk_pool.tile([128, S], bf16, tag="qt", name=f"qt_{p}")
                kt = qk_pool.tile([128, S], bf16, tag="kt", name=f"kt_{p}")
                QK[p] = (qt, kt)

            # ---------------- filler quanta ----------------
            uid = [0]

            def proj_quantum(dst_ap, w_ap, x_ap, use_st_slot=False):
                """dst_ap [128,512]bf16 <- sum_kc w_ap[:,kc,:].T @ x_ap[:,kc,:]"""
                uid[0] += 1
                if use_st_slot:
                    ps = st_pool.tile([128, 2, 512], f32, tag="st",
                                      name=f"prj{uid[0]}")
                    pslice = ps[:, 0, :]
                else:
                    ps = sh_pool.tile([128, 512], f32, tag="sh",
                                      name=f"prj{uid[0]}")
                    pslice = ps[:]
                for kc in range(KC):
                    nc.tensor.matmul(pslice, lhsT=w_ap[:, kc, :],
                                     rhs=x_ap[:, kc, :],
                                     start=(kc == 0), stop=(kc == KC - 1))
                nc.vector.tensor_copy(dst_ap, pslice)

            def v_quantum(sc, sbl):
                kb = sc * 4 + sbl
                uid[0] += 1
                ps = sh_pool.tile([128, 512], f32, tag="sh", name=f"v{kb}")
                for kc in range(KC):
                    nc.tensor.matmul(
                        ps[:], lhsT=XS[sc][:, kc, sbl * 128:(sbl + 1) * 128],
                        rhs=WV[:, kc, :],
                        start=(kc == 0), stop=(kc == KC - 1))
                nc.vector.tensor_copy(Vt[:, kb, :], ps[:])

            def qk_fillers(p):
                alloc_qk(p)
                qt, kt = QK[p]
                w = WQK[p]
                thunks = []
                for sc in range(NSC):
                    thunks.append(lambda sc=sc: proj_quantum(
                        kt[:, sc * 512:(sc + 1) * 512], w[:, 1], XS[sc]))
                for sc in range(NSC):
                    thunks.append(lambda sc=sc: proj_quantum(
                        qt[:, sc * 512:(sc + 1) * 512], w[:, 0], XS[sc]))
                return thunks

            def cproj_quantum(qc, qb, nn):
                ps = sh_pool.tile([128, 512], f32, tag="sh",
                                  name=f"cp{qc}_{qb}_{nn}")
                for p in range(NPAIR):
                    nc.tensor.matmul(
                        ps[:],
                        lhsT=ATN[:, p, qc * 512 + qb * 128:
                                 qc * 512 + (qb + 1) * 128],
                        rhs=WP[:, p, nn * 512:(nn + 1) * 512],
                        start=(p == 0), stop=(p == NPAIR - 1))
                ot = out_pool.tile([128, 512], f32, tag="ot",
                                   name=f"o{qc}_{qb}_{nn}")
                nc.vector.tensor_copy(ot[:], ps[:])
                r0 = qc * 512 + qb * 128
                nc.sync.dma_start(
                    out[r0:r0 + 128, nn * 512:(nn + 1) * 512], ot[:])

            def cproj_fillers(qc):
                return [lambda qb=qb, nn=nn: cproj_quantum(qc, qb, nn)
                        for qb in range(4) for nn in range(2)]

            # ---------------- prologue ----------------
            # KT pair0 fully + QT pair0 chunk 0; alternate the psum target
            # between the shared bank and an st-ring slot so the chains
            # double-buffer against their DVE casts. V rides as the first
            # block's fillers (one V block per kb step, AV lags by LAG).
            alloc_qk(0)
            qt0, kt0 = QK[0]
            chains = []
            for sc in range(NSC):
                chains.append((kt0[:, sc * 512:(sc + 1) * 512],
                               WQK[0][:, 1], XS[sc]))
            chains.append((qt0[:, 0:512], WQK[0][:, 0], XS[0]))
            for i, (dst, w, x) in enumerate(chains):
                proj_quantum(dst, w, x, use_st_slot=(i % 2 == 1))

            # ---------------- per-pair filler queues ----------------
            # Emission deadlines (Tile deps follow emission order!):
            #   V(kb)   before av_step(kb) of block (p0,qc0)  -> slot kb
            #   qt0[sc] before scores of block (p0,qc=sc)
            #   QK(p+1) fully before pair p+1's first block
            #   cproj(qc) only after boundary(3,qc) was emitted
            early_qt0 = [
                lambda sc=sc: proj_quantum(
                    qt0[:, sc * 512:(sc + 1) * 512], WQK[0][:, 0], XS[sc])
                for sc in range(1, NSC)]
            v_all = [lambda sc=sc, sbl=sbl: v_quantum(sc, sbl)
                     for sc in range(NSC) for sbl in range(4)]
            filler_q = {
                0: v_all + early_qt0 + qk_fillers(1),
                1: qk_fillers(2),
                2: qk_fillers(3),
                3: [],
            }
            # filler cadence per pair: p0 every step (V deadline), p1/p2
            # sparse (spread PE load), p3 every other step (8 cproj quanta)
            cadence = {0: 1, 1: 4, 2: 4, 3: 2}

            # ---------------- boundary ----------------
            def emit_boundary(pend):
                p, qc, avs, accA, accB = pend
                dpsb = sh_pool.tile([128, 512], f32, tag="sh",
                                    name=f"dps_{p}_{qc}")
                for h in range(2):
                    osl = dpsb[h * 64:(h + 1) * 64, :]
                    nc.tensor.matmul(osl, lhsT=ones[:], rhs=accA[:, h, :],
                                     start=True, stop=False,
                                     tile_position=(0, h * 64),
                                     skip_group_check=True)
                    nc.tensor.matmul(osl, lhsT=ones[:], rhs=accB[:, h, :],
                                     start=False, stop=True,
                                     tile_position=(0, h * 64),
                                     skip_group_check=True)
                inv = inv_pool.tile([128, 512], f32, tag="inv",
                                    name=f"inv_{p}_{qc}")
                nc.vector.reciprocal_approx_fast(inv[:], dpsb[:])
                nc.vector.tensor_mul(
                    ATN[:, p, qc * 512:(qc + 1) * 512], avs[:], inv[:])

            # ---------------- main attention blocks ----------------
            pending = None
            for p in range(NPAIR):
                QTp, KTp = QK[p]
                fq = filler_q[p]
                fi = [0]

                def filler_step(fq=fq, fi=fi):
                    if fi[0] < len(fq):
                        fq[fi[0]]()
                        fi[0] += 1

                for qc in range(NQC):
                    qsl = slice(qc * 512, (qc + 1) * 512)
                    avs = av_pool.tile([128, 512], f32, tag="avs",
                                       name=f"avs_{p}_{qc}")
                    accA = acc_pool.tile([128, 2, 512], bf16, tag="accA",
                                         name=f"accA_{p}_{qc}")
                    accB = acc_pool.tile([128, 2, 512], bf16, tag="accB",
                                         name=f"accB_{p}_{qc}")
                    pts = {}

                    def scores_step(kb, p=p, qc=qc, QTp=QTp, KTp=KTp,
                                    accA=accA, accB=accB, pts=pts, qsl=qsl):
                        st = st_pool.tile([128, 2, 512], f32, tag="st",
                                          name=f"st_{p}_{qc}_{kb}")
                        ksl = slice(kb * 128, (kb + 1) * 128)
                        for h in range(2):
                            hsl = slice(64 * h, 64 * h + 64)
                            nc.tensor.matmul(st[:, h, :],
                                             lhsT=KTp[hsl, ksl],
                                             rhs=QTp[hsl, qsl],
                                             start=True, stop=True)
                        pt = pt_pool.tile([128, 2, 512], bf16, tag="pt",
                                          name=f"pt_{p}_{qc}_{kb}")
                        nc.scalar.activation(pt[:], st[:], Exp, scale=SCALE)
                        pts[kb] = pt
                        acc = accA if kb % 2 == 0 else accB
                        if kb < 2:
                            nc.vector.tensor_copy(acc[:], pt[:])
                        else:
                            nc.vector.tensor_add(acc[:], acc[:], pt[:])

                    def av_step(kb, p=p, avs=avs, pts=pts):
                        pt = pts.pop(kb)
                        for h in range(2):
                            nc.tensor.matmul(
                                avs[64 * h:64 * h + 64, :],
                                lhsT=Vt[:, kb, p * 128 + 64 * h:
                                        p * 128 + 64 * h + 64],
                                rhs=pt[:, h, :],
                                start=(kb == 0), stop=(kb == KB - 1),
                                tile_position=(0, 64 * h),
                                skip_group_check=True)

                    # scores lead AV by LAG steps; boundary of the previous
                    # block rides after the first two score steps
                    cad = cadence[p]
                    for kb in range(LAG):
                        scores_step(kb)
                        if kb == 1 and pending is not None:
                            emit_boundary(pending)
                            pending = None
                            # c_proj fillers for the previous q-chunk may
                            # only be emitted after its boundary wrote ATN
                            if p == 3 and qc > 0:
                                fq.extend(cproj_fillers(qc - 1))
                        if kb % cad == 0:
                            filler_step()
                    for kb in range(LAG, KB):
                        scores_step(kb)
                        av_step(kb - LAG)
                        if kb % cad == 0:
                            filler_step()
                    for kb in range(KB - LAG, KB):
                        av_step(kb)
                    filler_step()
                    pending = (p, qc, avs, accA, accB)

                # drain remaining fillers before the next pair needs them
                while fi[0] < len(fq):
                    fq[fi[0]]()
                    fi[0] += 1

            # tail: last boundary + final c_proj chunk
            lastacc = pending[3]
            emit_boundary(pending)
            for q in cproj_fillers(NQC - 1):
                q()
            if dbg_on:
                qtd, ktd = QK[0]
                nc.sync.dma_start(dqt[:, :], qtd[:, :])
                nc.sync.dma_start(dkt[:, :], ktd[:, :])
                nc.sync.dma_start(dvt[:, :, :], Vt[:, :, :])
                nc.sync.dma_start(datn[:, :, :], ATN[:, :, :])
                nc.sync.dma_start(dacc[:, :, :], lastacc[:, :, :])

    nc.compile()
    return nc


def _get_nc():
    if "nc" not in _CACHED:
        _CACHED["nc"] = _build()
    return _CACHED["nc"]


def _shard(x, W_attn, W_proj):
    """Build per-core input maps with pre-swizzled bf16 layouts."""
    bf = ml_dtypes.bfloat16
    x = np.asarray(x, dtype=np.float32)
    W_attn = np.asarray(W_attn, dtype=np.float32)
    W_proj = np.asarray(W_proj, dtype=np.float32)
    in_maps = []
    for c in range(N_CORES):
        b, g = c // 2, c % 2
        # xs[sc, part, kc, j] = x[b, sc*512+j, kc*128+part]
        xt = x[b].T                                  # [D, S]
        xs_ = xt.reshape(KC, 128, NSC, 512).transpose(2, 1, 0, 3)
        # wqk[p, part, t, kc, f] = W_attn[kc*128+part, t*D + g*FC + p*128+f]
        # (partition dim second so the [128,2,KC,128] SBUF tile DMA is a
        # straight linear copy)
        wqk_ = np.empty((NPAIR, 128, 2, KC, 128), dtype=bf)
        for t in range(2):
            wslab = W_attn[:, t * D + g * FC: t * D + (g + 1) * FC]  # [D,FC]
            wr = wslab.reshape(KC, 128, NPAIR, 128).transpose(2, 1, 0, 3)
            wqk_[:, :, t] = wr.astype(bf)
        wv_ = W_attn[:, 2 * D + g * FC: 2 * D + (g + 1) * FC]        # [D,FC]
        wv_ = wv_.reshape(KC, 128, FC).transpose(1, 0, 2).astype(bf)
        # wp[part, p, m] = W_proj[g*FC + p*128 + part, m]
        wp_ = W_proj[g * FC:(g + 1) * FC, :].reshape(NPAIR, 128, D)
        wp_ = wp_.transpose(1, 0, 2).astype(bf)
        in_maps.append({
            "xs": np.ascontiguousarray(xs_.astype(bf)),
            "wqk": np.ascontiguousarray(wqk_),
            "wv": np.ascontiguousarray(wv_),
            "wp": np.ascontiguousarray(wp_),
        })
    return in_maps


def kernel(x, W_attn, W_proj):
    from concourse.bass_utils import run_bass_kernel_spmd

    nc = _get_nc()
    in_maps = _shard(x, W_attn, W_proj)
    trace = os.environ.get("BASS_PROBLEM_TRACE", "0") == "1"
    res = run_bass_kernel_spmd(nc, in_maps, list(range(N_CORES)), trace=trace)
    _CACHED["last_result"] = res
    out = np.empty((B, S, D), dtype=np.float32)
    for b in range(B):
        out[b] = res.results[2 * b]["out"] + res.results[2 * b + 1]["out"]
    return out


# revision 12
# speedup vs baseline: 1.3791x; 1.2380x over previous
"""Trainium2 Bass kernel for nn_MultiHeadAttention (B=4, S=2048, D=1024, H=16).

Sharding: 8 cores = 4 batches x 2 head-groups. Core c handles batch c//2,
heads [8*(c%2), 8*(c%2)+8). Host sums the two c_proj partials per batch.

v2 design (from trace analysis of the 503us baseline):
  - all matmul operands bf16 (FWL weight loads, half DMA bytes)
  - x^T resident in SBUF (loaded once, reused by all pairs' QK chains)
  - scores PSUM ring of 3 tiles [128,2h,512q] absorbs scheduling jitter so
    the ScalarE exp stream (the ~280us floor) never stalls
  - software-pipelined emission: scores lead AV by 4 steps so the PE queue
    never head-of-line blocks the exp stream
  - denominator: bf16 A/B accumulation chains on DVE; at block boundary a
    ones[128,64]-weighted matmul replicates sum-over-keys across all
    partitions (h0 -> psum rows 0:64, h1 -> 64:128, col-tiled), one DVE
    reciprocal + one tensor_mul normalizes avs -> ATN (no gpsimd bcast)
  - pair-outer / qc-inner(512) blocks; QK(p+1), V, and c_proj(qc) run as
    small PE filler quanta inside the block streams; c_proj overlaps the
    pair-3 blocks so the tail is one q-chunk
"""

import contextlib
import ctypes
import os
import sys
import types

import numpy as np
import ml_dtypes

# ---------------------------------------------------------------------------
# NTFF profiling hook (used when BASS_PROBLEM_TRACE=1)
# ---------------------------------------------------------------------------
_AXON_SO = "/opt/axon/libaxon_pjrt.so"


def _install_ntff_hook():
    if "antenv.axon_hooks" in sys.modules:
        return
    try:
        import antenv
    except ImportError:
        return
    try:
        lib = ctypes.CDLL(_AXON_SO)
    except OSError:
        return
    if not hasattr(lib, "axon_start_nrt_profile"):
        return
    lib.axon_start_nrt_profile.argtypes = [
        ctypes.POINTER(ctypes.c_int64),
        ctypes.c_size_t,
    ]
    lib.axon_start_nrt_profile.restype = ctypes.c_int64
    lib.axon_stop_nrt_profile.argtypes = [ctypes.c_char_p]
    lib.axon_stop_nrt_profile.restype = ctypes.c_int64

    @contextlib.contextmanager
    def _hook(output_dir, device_ids):
        import jax

        jax.devices()
        if device_ids:
            ids = (ctypes.c_int64 * len(device_ids))(*device_ids)
            rc = lib.axon_start_nrt_profile(ids, len(device_ids))
        else:
            rc = lib.axon_start_nrt_profile(None, 0)
        if rc != 0:
            raise RuntimeError(f"axon_start_nrt_profile rc={rc}")
        try:
            yield
        finally:
            n = lib.axon_stop_nrt_profile(str(output_dir).encode())
            print(f"profile: {n} file(s) written to {output_dir}", file=sys.stderr)

    mod = types.ModuleType("antenv.axon_hooks")
    holder = [_hook]
    mod.get_axon_ntff_profile_hook = lambda: holder[0]
    mod.set_axon_ntff_profile_hook = lambda h: holder.__setitem__(0, h)
    sys.modules["antenv.axon_hooks"] = mod
    antenv.axon_hooks = mod


_install_ntff_hook()

# ---------------------------------------------------------------------------
# Problem constants (hardcoded per the contract)
# ---------------------------------------------------------------------------
B, S, D = 4, 2048, 1024
H, DK = 16, 64
N_CORES = 8
HPC = 8            # heads per core
NPAIR = HPC // 2   # head pairs per core = 4
FC = HPC * DK      # features per core = 512
SCALE = 1.0 / float(np.sqrt(DK))  # 0.125

KC = D // 128      # 8 contraction chunks for qkv projections
NSC = 4            # seq chunks of 512 for x / QK tiles
KB = S // 128      # 16 key blocks
NQC = 4            # q chunks of 512
LAG = 4            # AV trails scores by this many kb steps

_CACHED = {}


def _build():
    import concourse.tile as tile
    from concourse import bacc, mybir

    f32 = mybir.dt.float32
    bf16 = mybir.dt.bfloat16
    Exp = mybir.ActivationFunctionType.Exp

    nc = bacc.Bacc("TRN2", target_bir_lowering=False, debug=False,
                   num_devices=N_CORES)

    # Pre-swizzled DRAM inputs (host packs these; contiguous per partition)
    xs = nc.dram_tensor("xs", [NSC, 128, KC, 512], bf16,
                        kind="ExternalInput").ap()
    wqk = nc.dram_tensor("wqk", [NPAIR, 128, 2, KC, 128], bf16,
                         kind="ExternalInput").ap()
    wv = nc.dram_tensor("wv", [128, KC, FC], bf16, kind="ExternalInput").ap()
    wp = nc.dram_tensor("wp", [128, NPAIR, D], bf16,
                        kind="ExternalInput").ap()
    out = nc.dram_tensor("out", [S, D], f32, kind="ExternalOutput").ap()
    dbg_on = os.environ.get("BASS_DEBUG_DUMP", "0") == "1"
    if dbg_on:
        dqt = nc.dram_tensor("dqt", [128, S], bf16, kind="ExternalOutput").ap()
        dkt = nc.dram_tensor("dkt", [128, S], bf16, kind="ExternalOutput").ap()
        dvt = nc.dram_tensor("dvt", [128, KB, FC], bf16,
                             kind="ExternalOutput").ap()
        datn = nc.dram_tensor("datn", [128, NPAIR, S], bf16,
                              kind="ExternalOutput").ap()
        dacc = nc.dram_tensor("dacc", [128, 2, 512], bf16,
                              kind="ExternalOutput").ap()

    with tile.TileContext(nc) as tc:
        with (
            tc.tile_pool(name="xsp", bufs=NSC) as xs_pool,
            tc.tile_pool(name="wqkp", bufs=NPAIR) as wqk_pool,
            tc.tile_pool(name="wvp", bufs=1) as wv_pool,
            tc.tile_pool(name="wpp", bufs=1) as wp_pool,
            tc.tile_pool(name="qkp", bufs=2) as qk_pool,
            tc.tile_pool(name="vtp", bufs=1) as v_pool,
            tc.tile_pool(name="ptp", bufs=16) as pt_pool,
            tc.tile_pool(name="accp", bufs=2) as acc_pool,
            tc.tile_pool(name="invp", bufs=2) as inv_pool,
            tc.tile_pool(name="atnp", bufs=1) as atn_pool,
            tc.tile_pool(name="outp", bufs=3) as out_pool,
            tc.tile_pool(name="cstp", bufs=1) as cst_pool,
            tc.tile_pool(name="stp", bufs=2, space="PSUM") as st_pool,
            tc.tile_pool(name="avp", bufs=2, space="PSUM") as av_pool,
            tc.tile_pool(name="shp", bufs=2, space="PSUM") as sh_pool,
        ):
            # ---------------- static tiles ----------------
            ones = cst_pool.tile([128, 64], bf16, tag="ones")
            nc.gpsimd.memset(ones[:], 1.0)

            XS = [xs_pool.tile([128, KC, 512], bf16, tag="xs",
                               name=f"xs_{sc}") for sc in range(NSC)]
            WV = wv_pool.tile([128, KC, FC], bf16, tag="wv")
            WP = wp_pool.tile([128, NPAIR, D], bf16, tag="wp")
            Vt = v_pool.tile([128, KB, FC], bf16, tag="vt")
            ATN = atn_pool.tile([128, NPAIR, S], bf16, tag="atn")

            WQK = []
            for p in range(NPAIR):
                t = wqk_pool.tile([128, 2, KC, 128], bf16, tag="wqk",
                                  name=f"wqk_{p}")
                WQK.append(t)

            # DMA order: pair0 weights + x chunk 0 first (prologue critical
            # path), then everything else.
            nc.sync.dma_start(WQK[0][:], wqk[0])
            nc.sync.dma_start(XS[0][:], xs[0])
            nc.sync.dma_start(WV[:], wv[:])
            for sc in range(1, NSC):
                nc.sync.dma_start(XS[sc][:], xs[sc])
            for p in range(1, NPAIR):
                nc.sync.dma_start(WQK[p][:], wqk[p])
            nc.sync.dma_start(WP[:], wp[:])

            # QT/KT tiles per pair (ring of 2)
            QK = {}

            def alloc_qk(p):
                qt = qk_pool.tile([128, S], bf16, tag="qt", name=f"qt_{p}")
                kt = qk_pool.tile([128, S], bf16, tag="kt", name=f"kt_{p}")
                QK[p] = (qt, kt)

            # ---------------- filler quanta ----------------
            uid = [0]

            def proj_quantum(dst_ap, w_ap, x_ap):
                """dst_ap [128,512]bf16 <- sum_kc w_ap[:,kc,:].T @ x_ap[:,kc,:]"""
                uid[0] += 1
                ps = sh_pool.tile([128, 512], f32, tag="sh",
                                  name=f"prj{uid[0]}")
                pslice = ps[:]
                for kc in range(KC):
                    nc.tensor.matmul(pslice, lhsT=w_ap[:, kc, :],
                                     rhs=x_ap[:, kc, :],
                                     start=(kc == 0), stop=(kc == KC - 1))
                nc.vector.tensor_copy(dst_ap, pslice)

            def v_quantum(sc, sbl):
                kb = sc * 4 + sbl
                uid[0] += 1
                ps = sh_pool.tile([128, 512], f32, tag="sh", name=f"v{kb}")
                for kc in range(KC):
                    nc.tensor.matmul(
                        ps[:], lhsT=XS[sc][:, kc, sbl * 128:(sbl + 1) * 128],
                        rhs=WV[:, kc, :],
                        start=(kc == 0), stop=(kc == KC - 1))
                nc.vector.tensor_copy(Vt[:, kb, :], ps[:])

            def qk_fillers(p):
                alloc_qk(p)
                qt, kt = QK[p]
                w = WQK[p]
                thunks = []
                for sc in range(NSC):
                    thunks.append(lambda sc=sc: proj_quantum(
                        kt[:, sc * 512:(sc + 1) * 512], w[:, 1], XS[sc]))
                for sc in range(NSC):
                    thunks.append(lambda sc=sc: proj_quantum(
                        qt[:, sc * 512:(sc + 1) * 512], w[:, 0], XS[sc]))
                return thunks

            def cproj_quantum(qc, qb, nn):
                ps = sh_pool.tile([128, 512], f32, tag="sh",
                                  name=f"cp{qc}_{qb}_{nn}")
                for p in range(NPAIR):
                    nc.tensor.matmul(
                        ps[:],
                        lhsT=ATN[:, p, qc * 512 + qb * 128:
                                 qc * 512 + (qb + 1) * 128],
                        rhs=WP[:, p, nn * 512:(nn + 1) * 512],
                        start=(p == 0), stop=(p == NPAIR - 1))
                ot = out_pool.tile([128, 512], f32, tag="ot",
                                   name=f"o{qc}_{qb}_{nn}")
                nc.vector.tensor_copy(ot[:], ps[:])
                r0 = qc * 512 + qb * 128
                nc.sync.dma_start(
                    out[r0:r0 + 128, nn * 512:(nn + 1) * 512], ot[:])

            def cproj_fillers(qc):
                return [lambda qb=qb, nn=nn: cproj_quantum(qc, qb, nn)
                        for qb in range(4) for nn in range(2)]

            # ---------------- prologue ----------------
            # KT pair0 fully + QT pair0 chunk 0; alternate the psum target
            # between the shared bank and an st-ring slot so the chains
            # double-buffer against their DVE casts. V rides as the first
            # block's fillers (one V block per kb step, AV lags by LAG).
            alloc_qk(0)
            qt0, kt0 = QK[0]
            chains = []
            for sc in range(NSC):
                chains.append((kt0[:, sc * 512:(sc + 1) * 512],
                               WQK[0][:, 1], XS[sc]))
            chains.append((qt0[:, 0:512], WQK[0][:, 0], XS[0]))
            for dst, w, x in chains:
                proj_quantum(dst, w, x)

            # ---------------- per-pair filler queues ----------------
            # Emission deadlines (Tile deps follow emission order!):
            #   V(kb)   before av_step(kb) of block (p0,qc0)  -> slot kb
            #   qt0[sc] before scores of block (p0,qc=sc)
            #   QK(p+1) fully before pair p+1's first block
            #   cproj(qc) only after boundary(3,qc) was emitted
            early_qt0 = [
                lambda sc=sc: proj_quantum(
                    qt0[:, sc * 512:(sc + 1) * 512], WQK[0][:, 0], XS[sc])
                for sc in range(1, NSC)]
            v_all = [lambda sc=sc, sbl=sbl: v_quantum(sc, sbl)
                     for sc in range(NSC) for sbl in range(4)]
            # (queue order below: early0 first so qc1's QT chunk jumps the
            # chain queue, then all V blocks (emitted at step kb, consumed
            # by av(kb) at step kb+LAG), then the rest)
            filler_q = {
                0: v_all + early_qt0 + qk_fillers(1),
                1: qk_fillers(2),
                2: qk_fillers(3),
                3: [],
            }
            # filler cadence per pair: p0 every step (V deadline), p1/p2
            # sparse (spread PE load), p3 every other step (8 cproj quanta)
            cadence = {0: 1, 1: 4, 2: 4, 3: 2}

            # ---------------- boundary ----------------
            def emit_boundary(pend):
                p, qc, avs, accA, accB = pend
                dpsb = sh_pool.tile([128, 512], f32, tag="sh",
                                    name=f"dps_{p}_{qc}")
                for h in range(2):
                    osl = dpsb[h * 64:(h + 1) * 64, :]
                    nc.tensor.matmul(osl, lhsT=ones[:], rhs=accA[:, h, :],
                                     start=True, stop=False,
                                     tile_position=(0, h * 64),
                                     skip_group_check=True)
                    nc.tensor.matmul(osl, lhsT=ones[:], rhs=accB[:, h, :],
                                     start=False, stop=True,
                                     tile_position=(0, h * 64),
                                     skip_group_check=True)
                inv = inv_pool.tile([128, 512], f32, tag="inv",
                                    name=f"inv_{p}_{qc}")
                nc.vector.reciprocal_approx_fast(inv[:], dpsb[:])
                nc.vector.tensor_mul(
                    ATN[:, p, qc * 512:(qc + 1) * 512], avs[:], inv[:])

            # ---------------- main attention blocks ----------------
            pending = None
            for p in range(NPAIR):
                QTp, KTp = QK[p]
                fq = filler_q[p]
                fi = [0]

                def filler_step(fq=fq, fi=fi):
                    if fi[0] < len(fq):
                        fq[fi[0]]()
                        fi[0] += 1

                for qc in range(NQC):
                    qsl = slice(qc * 512, (qc + 1) * 512)
                    avs = av_pool.tile([128, 512], f32, tag="avs",
                                       name=f"avs_{p}_{qc}")
                    accA = acc_pool.tile([128, 2, 512], bf16, tag="accA",
                                         name=f"accA_{p}_{qc}")
                    accB = acc_pool.tile([128, 2, 512], bf16, tag="accB",
                                         name=f"accB_{p}_{qc}")
                    pts = {}

                    def scores_step(kb, p=p, qc=qc, QTp=QTp, KTp=KTp,
                                    accA=accA, accB=accB, pts=pts, qsl=qsl):
                        st = st_pool.tile([128, 2, 512], f32, tag="st",
                                          name=f"st_{p}_{qc}_{kb}")
                        ksl = slice(kb * 128, (kb + 1) * 128)
                        pt = pt_pool.tile([128, 2, 512], bf16, tag="pt",
                                          name=f"pt_{p}_{qc}_{kb}")
                        with tc.high_priority(offset=100000):
                            for h in range(2):
                                hsl = slice(64 * h, 64 * h + 64)
                                nc.tensor.matmul(st[:, h, :],
                                                 lhsT=KTp[hsl, ksl],
                                                 rhs=QTp[hsl, qsl],
                                                 start=True, stop=True)
                            nc.scalar.activation(pt[:], st[:], Exp,
                                                 scale=SCALE)
                        pts[kb] = pt
                        acc = accA if kb % 2 == 0 else accB
                        if kb < 2:
                            nc.vector.tensor_copy(acc[:], pt[:])
                        else:
                            nc.vector.tensor_add(acc[:], acc[:], pt[:])

                    def av_step(kb, p=p, avs=avs, pts=pts):
                        pt = pts.pop(kb)
                        for h in range(2):
                            nc.tensor.matmul(
                                avs[64 * h:64 * h + 64, :],
                                lhsT=Vt[:, kb, p * 128 + 64 * h:
                                        p * 128 + 64 * h + 64],
                                rhs=pt[:, h, :],
                                start=(kb == 0), stop=(kb == KB - 1),
                                tile_position=(0, 64 * h),
                                skip_group_check=True)

                    # scores lead AV by LAG steps; boundary of the previous
                    # block rides after the first two score steps
                    cad = cadence[p]
                    for kb in range(LAG):
                        scores_step(kb)
                        if kb == 1 and pending is not None:
                            emit_boundary(pending)
                            pending = None
                            # c_proj fillers for the previous q-chunk may
                            # only be emitted after its boundary wrote ATN
                            if p == 3 and qc > 0:
                                fq.extend(cproj_fillers(qc - 1))
                        if kb % cad == 0:
                            filler_step()
                    for kb in range(LAG, KB):
                        scores_step(kb)
                        av_step(kb - LAG)
                        if kb % cad == 0:
                            filler_step()
                    for kb in range(KB - LAG, KB):
                        av_step(kb)
                    filler_step()
                    pending = (p, qc, avs, accA, accB)

                # drain remaining fillers before the next pair needs them
                while fi[0] < len(fq):
                    fq[fi[0]]()
                    fi[0] += 1

            # tail: last boundary + final c_proj chunk
            lastacc = pending[3]
            emit_boundary(pending)
            for q in cproj_fillers(NQC - 1):
                q()
            if dbg_on:
                qtd, ktd = QK[0]
                nc.sync.dma_start(dqt[:, :], qtd[:, :])
                nc.sync.dma_start(dkt[:, :], ktd[:, :])
                nc.sync.dma_start(dvt[:, :, :], Vt[:, :, :])
                nc.sync.dma_start(datn[:, :, :], ATN[:, :, :])
                nc.sync.dma_start(dacc[:, :, :], lastacc[:, :, :])

    nc.compile()
    return nc


def _get_nc():
    if "nc" not in _CACHED:
        _CACHED["nc"] = _build()
    return _CACHED["nc"]


def _shard(x, W_attn, W_proj):
    """Build per-core input maps with pre-swizzled bf16 layouts."""
    bf = ml_dtypes.bfloat16
    x = np.asarray(x, dtype=np.float32)
    W_attn = np.asarray(W_attn, dtype=np.float32)
    W_proj = np.asarray(W_proj, dtype=np.float32)
    in_maps = []
    for c in range(N_CORES):
        b, g = c // 2, c % 2
        # xs[sc, part, kc, j] = x[b, sc*512+j, kc*128+part]
        xt = x[b].T                                  # [D, S]
        xs_ = xt.reshape(KC, 128, NSC, 512).transpose(2, 1, 0, 3)
        # wqk[p, part, t, kc, f] = W_attn[kc*128+part, t*D + g*FC + p*128+f]
        # (partition dim second so the [128,2,KC,128] SBUF tile DMA is a
        # straight linear copy)
        wqk_ = np.empty((NPAIR, 128, 2, KC, 128), dtype=bf)
        for t in range(2):
            wslab = W_attn[:, t * D + g * FC: t * D + (g + 1) * FC]  # [D,FC]
            wr = wslab.reshape(KC, 128, NPAIR, 128).transpose(2, 1, 0, 3)
            wqk_[:, :, t] = wr.astype(bf)
        wv_ = W_attn[:, 2 * D + g * FC: 2 * D + (g + 1) * FC]        # [D,FC]
        wv_ = wv_.reshape(KC, 128, FC).transpose(1, 0, 2).astype(bf)
        # wp[part, p, m] = W_proj[g*FC + p*128 + part, m]
        wp_ = W_proj[g * FC:(g + 1) * FC, :].reshape(NPAIR, 128, D)
        wp_ = wp_.transpose(1, 0, 2).astype(bf)
        in_maps.append({
            "xs": np.ascontiguousarray(xs_.astype(bf)),
            "wqk": np.ascontiguousarray(wqk_),
            "wv": np.ascontiguousarray(wv_),
            "wp": np.ascontiguousarray(wp_),
        })
    return in_maps


def kernel(x, W_attn, W_proj):
    from concourse.bass_utils import run_bass_kernel_spmd

    nc = _get_nc()
    in_maps = _shard(x, W_attn, W_proj)
    trace = os.environ.get("BASS_PROBLEM_TRACE", "0") == "1"
    res = run_bass_kernel_spmd(nc, in_maps, list(range(N_CORES)), trace=trace)
    _CACHED["last_result"] = res
    out = np.empty((B, S, D), dtype=np.float32)
    for b in range(B):
        out[b] = res.results[2 * b]["out"] + res.results[2 * b + 1]["out"]
    return out


# revision 16
# speedup vs baseline: 1.4055x; 1.0191x over previous
"""Trainium2 Bass kernel for nn_MultiHeadAttention (B=4, S=2048, D=1024, H=16).

Sharding: 8 cores = 4 batches x 2 head-groups. Core c handles batch c//2,
heads [8*(c%2), 8*(c%2)+8). Host sums the two c_proj partials per batch.

v2 design (from trace analysis of the 503us baseline):
  - all matmul operands bf16 (FWL weight loads, half DMA bytes)
  - x^T resident in SBUF (loaded once, reused by all pairs' QK chains)
  - scores PSUM ring of 3 tiles [128,2h,512q] absorbs scheduling jitter so
    the ScalarE exp stream (the ~280us floor) never stalls
  - software-pipelined emission: scores lead AV by 4 steps so the PE queue
    never head-of-line blocks the exp stream
  - denominator: bf16 A/B accumulation chains on DVE; at block boundary a
    ones[128,64]-weighted matmul replicates sum-over-keys across all
    partitions (h0 -> psum rows 0:64, h1 -> 64:128, col-tiled), one DVE
    reciprocal + one tensor_mul normalizes avs -> ATN (no gpsimd bcast)
  - pair-outer / qc-inner(512) blocks; QK(p+1), V, and c_proj(qc) run as
    small PE filler quanta inside the block streams; c_proj overlaps the
    pair-3 blocks so the tail is one q-chunk
"""

import contextlib
import ctypes
import os
import sys
import types

import numpy as np
import ml_dtypes

# ---------------------------------------------------------------------------
# NTFF profiling hook (used when BASS_PROBLEM_TRACE=1)
# ---------------------------------------------------------------------------
_AXON_SO = "/opt/axon/libaxon_pjrt.so"


def _install_ntff_hook():
    if "antenv.axon_hooks" in sys.modules:
        return
    try:
        import antenv
    except ImportError:
        return
    try:
        lib = ctypes.CDLL(_AXON_SO)
    except OSError:
        return
    if not hasattr(lib, "axon_start_nrt_profile"):
        return
    lib.axon_start_nrt_profile.argtypes = [
        ctypes.POINTER(ctypes.c_int64),
        ctypes.c_size_t,
    ]
    lib.axon_start_nrt_profile.restype = ctypes.c_int64
    lib.axon_stop_nrt_profile.argtypes = [ctypes.c_char_p]
    lib.axon_stop_nrt_profile.restype = ctypes.c_int64

    @contextlib.contextmanager
    def _hook(output_dir, device_ids):
        import jax

        jax.devices()
        if device_ids:
            ids = (ctypes.c_int64 * len(device_ids))(*device_ids)
            rc = lib.axon_start_nrt_profile(ids, len(device_ids))
        else:
            rc = lib.axon_start_nrt_profile(None, 0)
        if rc != 0:
            raise RuntimeError(f"axon_start_nrt_profile rc={rc}")
        try:
            yield
        finally:
            n = lib.axon_stop_nrt_profile(str(output_dir).encode())
            print(f"profile: {n} file(s) written to {output_dir}", file=sys.stderr)

    mod = types.ModuleType("antenv.axon_hooks")
    holder = [_hook]
    mod.get_axon_ntff_profile_hook = lambda: holder[0]
    mod.set_axon_ntff_profile_hook = lambda h: holder.__setitem__(0, h)
    sys.modules["antenv.axon_hooks"] = mod
    antenv.axon_hooks = mod


_install_ntff_hook()

# ---------------------------------------------------------------------------
# Problem constants (hardcoded per the contract)
# ---------------------------------------------------------------------------
B, S, D = 4, 2048, 1024
H, DK = 16, 64
N_CORES = 8
HPC = 8            # heads per core
NPAIR = HPC // 2   # head pairs per core = 4
FC = HPC * DK      # features per core = 512
SCALE = 1.0 / float(np.sqrt(DK))  # 0.125

KC = D // 128      # 8 contraction chunks for qkv projections
NSC = 4            # seq chunks of 512 for x / QK tiles
KB = S // 128      # 16 key blocks
NQC = 4            # q chunks of 512
LAG = 4            # AV trails scores by this many kb steps

_CACHED = {}


def _build():
    import concourse.tile as tile
    from concourse import bacc, mybir

    f32 = mybir.dt.float32
    bf16 = mybir.dt.bfloat16
    Exp = mybir.ActivationFunctionType.Exp

    nc = bacc.Bacc("TRN2", target_bir_lowering=False, debug=False,
                   num_devices=N_CORES)

    # Pre-swizzled DRAM inputs (host packs these; contiguous per partition)
    xs = nc.dram_tensor("xs", [NSC, 128, KC, 512], bf16,
                        kind="ExternalInput").ap()
    wqk = nc.dram_tensor("wqk", [NPAIR, 128, 2, KC, 128], bf16,
                         kind="ExternalInput").ap()
    wv = nc.dram_tensor("wv", [128, KC, FC], bf16, kind="ExternalInput").ap()
    wp = nc.dram_tensor("wp", [128, NPAIR, D], bf16,
                        kind="ExternalInput").ap()
    out = nc.dram_tensor("out", [S, D], f32, kind="ExternalOutput").ap()
    dbg_on = os.environ.get("BASS_DEBUG_DUMP", "0") == "1"
    if dbg_on:
        dqt = nc.dram_tensor("dqt", [128, S], bf16, kind="ExternalOutput").ap()
        dkt = nc.dram_tensor("dkt", [128, S], bf16, kind="ExternalOutput").ap()
        dvt = nc.dram_tensor("dvt", [128, KB, FC], bf16,
                             kind="ExternalOutput").ap()
        datn = nc.dram_tensor("datn", [128, NPAIR, S], bf16,
                              kind="ExternalOutput").ap()
        dacc = nc.dram_tensor("dacc", [128, 2, 512], bf16,
                              kind="ExternalOutput").ap()

    with tile.TileContext(nc) as tc:
        with (
            tc.tile_pool(name="xsp", bufs=NSC) as xs_pool,
            tc.tile_pool(name="wqkp", bufs=NPAIR) as wqk_pool,
            tc.tile_pool(name="wvp", bufs=1) as wv_pool,
            tc.tile_pool(name="wpp", bufs=1) as wp_pool,
            tc.tile_pool(name="qkp", bufs=2) as qk_pool,
            tc.tile_pool(name="vtp", bufs=1) as v_pool,
            tc.tile_pool(name="ptp", bufs=24) as pt_pool,
            tc.tile_pool(name="accp", bufs=2) as acc_pool,
            tc.tile_pool(name="invp", bufs=2) as inv_pool,
            tc.tile_pool(name="atnp", bufs=1) as atn_pool,
            tc.tile_pool(name="outp", bufs=3) as out_pool,
            tc.tile_pool(name="cstp", bufs=1) as cst_pool,
            tc.tile_pool(name="stp", bufs=2, space="PSUM") as st_pool,
            tc.tile_pool(name="avp", bufs=2, space="PSUM") as av_pool,
            tc.tile_pool(name="shp", bufs=2, space="PSUM") as sh_pool,
        ):
            # ---------------- static tiles ----------------
            ones = cst_pool.tile([128, 64], bf16, tag="ones")
            nc.gpsimd.memset(ones[:], 1.0)

            XS = [xs_pool.tile([128, KC, 512], bf16, tag="xs",
                               name=f"xs_{sc}") for sc in range(NSC)]
            WV = wv_pool.tile([128, KC, FC], bf16, tag="wv")
            WP = wp_pool.tile([128, NPAIR, D], bf16, tag="wp")
            Vt = v_pool.tile([128, KB, FC], bf16, tag="vt")
            ATN = atn_pool.tile([128, NPAIR, S], bf16, tag="atn")

            WQK = []
            for p in range(NPAIR):
                t = wqk_pool.tile([128, 2, KC, 128], bf16, tag="wqk",
                                  name=f"wqk_{p}")
                WQK.append(t)

            # DMA order: pair0 weights + x chunk 0 first (prologue critical
            # path), then everything else.
            for t in range(2):
                nc.sync.dma_start(WQK[0][:, t], wqk[0, :, t])
            for kc in range(KC):
                nc.sync.dma_start(XS[0][:, kc, :], xs[0, :, kc, :])
            for kc in range(0, KC, 2):
                nc.sync.dma_start(WV[:, kc:kc + 2, :], wv[:, kc:kc + 2, :])
            for sc in range(1, NSC):
                for kc in range(0, KC, 2):
                    nc.sync.dma_start(XS[sc][:, kc:kc + 2, :],
                                      xs[sc, :, kc:kc + 2, :])
            for p in range(1, NPAIR):
                nc.sync.dma_start(WQK[p][:], wqk[p])
            nc.sync.dma_start(WP[:], wp[:])

            # QT/KT tiles per pair (ring of 2)
            QK = {}

            def alloc_qk(p):
                qt = qk_pool.tile([128, S], bf16, tag="qt", name=f"qt_{p}")
                kt = qk_pool.tile([128, S], bf16, tag="kt", name=f"kt_{p}")
                QK[p] = (qt, kt)

            # ---------------- filler quanta ----------------
            uid = [0]

            def proj_quantum(dst_ap, w_ap, x_ap):
                """dst_ap [128,512]bf16 <- sum_kc w_ap[:,kc,:].T @ x_ap[:,kc,:]"""
                uid[0] += 1
                ps = sh_pool.tile([128, 512], f32, tag="sh",
                                  name=f"prj{uid[0]}")
                pslice = ps[:]
                for kc in range(KC):
                    nc.tensor.matmul(pslice, lhsT=w_ap[:, kc, :],
                                     rhs=x_ap[:, kc, :],
                                     start=(kc == 0), stop=(kc == KC - 1))
                nc.vector.tensor_copy(dst_ap, pslice)

            def v_quantum(sc, sbl):
                kb = sc * 4 + sbl
                uid[0] += 1
                ps = sh_pool.tile([128, 512], f32, tag="sh", name=f"v{kb}")
                for kc in range(KC):
                    nc.tensor.matmul(
                        ps[:], lhsT=XS[sc][:, kc, sbl * 128:(sbl + 1) * 128],
                        rhs=WV[:, kc, :],
                        start=(kc == 0), stop=(kc == KC - 1))
                nc.vector.tensor_copy(Vt[:, kb, :], ps[:])

            def qk_fillers(p):
                alloc_qk(p)
                qt, kt = QK[p]
                w = WQK[p]
                thunks = []
                for sc in range(NSC):
                    thunks.append(lambda sc=sc: proj_quantum(
                        kt[:, sc * 512:(sc + 1) * 512], w[:, 1], XS[sc]))
                for sc in range(NSC):
                    thunks.append(lambda sc=sc: proj_quantum(
                        qt[:, sc * 512:(sc + 1) * 512], w[:, 0], XS[sc]))
                return thunks

            def cproj_quantum(qc, qb, nn):
                ps = sh_pool.tile([128, 512], f32, tag="sh",
                                  name=f"cp{qc}_{qb}_{nn}")
                for p in range(NPAIR):
                    nc.tensor.matmul(
                        ps[:],
                        lhsT=ATN[:, p, qc * 512 + qb * 128:
                                 qc * 512 + (qb + 1) * 128],
                        rhs=WP[:, p, nn * 512:(nn + 1) * 512],
                        start=(p == 0), stop=(p == NPAIR - 1))
                ot = out_pool.tile([128, 512], f32, tag="ot",
                                   name=f"o{qc}_{qb}_{nn}")
                nc.vector.tensor_copy(ot[:], ps[:])
                r0 = qc * 512 + qb * 128
                nc.sync.dma_start(
                    out[r0:r0 + 128, nn * 512:(nn + 1) * 512], ot[:])

            def cproj_fillers(qc):
                return [lambda qb=qb, nn=nn: cproj_quantum(qc, qb, nn)
                        for qb in range(4) for nn in range(2)]

            # ---------------- prologue ----------------
            # KT pair0 fully + QT pair0 chunk 0; alternate the psum target
            # between the shared bank and an st-ring slot so the chains
            # double-buffer against their DVE casts. V rides as the first
            # block's fillers (one V block per kb step, AV lags by LAG).
            alloc_qk(0)
            qt0, kt0 = QK[0]
            chains = []
            for sc in range(NSC):
                chains.append((kt0[:, sc * 512:(sc + 1) * 512],
                               WQK[0][:, 1], XS[sc]))
            chains.append((qt0[:, 0:512], WQK[0][:, 0], XS[0]))
            for dst, w, x in chains:
                proj_quantum(dst, w, x)

            # ---------------- per-pair filler queues ----------------
            # Emission deadlines (Tile deps follow emission order!):
            #   V(kb)   before av_step(kb) of block (p0,qc0)  -> slot kb
            #   qt0[sc] before scores of block (p0,qc=sc)
            #   QK(p+1) fully before pair p+1's first block
            #   cproj(qc) only after boundary(3,qc) was emitted
            early_qt0 = [
                lambda sc=sc: proj_quantum(
                    qt0[:, sc * 512:(sc + 1) * 512], WQK[0][:, 0], XS[sc])
                for sc in range(1, NSC)]
            v_all = [lambda sc=sc, sbl=sbl: v_quantum(sc, sbl)
                     for sc in range(NSC) for sbl in range(4)]
            # (queue order below: early0 first so qc1's QT chunk jumps the
            # chain queue, then all V blocks (emitted at step kb, consumed
            # by av(kb) at step kb+LAG), then the rest)
            filler_q = {
                0: v_all + early_qt0 + qk_fillers(1),
                1: qk_fillers(2),
                2: qk_fillers(3),
                3: [],
            }
            # filler cadence per pair: p0 every step (V deadline), p1/p2
            # sparse (spread PE load), p3 every other step (8 cproj quanta)
            cadence = {0: 1, 1: 4, 2: 4, 3: 2}

            # ---------------- boundary ----------------
            def emit_boundary(pend):
                p, qc, avs, accA = pend
                dpsb = sh_pool.tile([128, 512], f32, tag="sh",
                                    name=f"dps_{p}_{qc}")
                for h in range(2):
                    osl = dpsb[h * 64:(h + 1) * 64, :]
                    nc.tensor.matmul(osl, lhsT=ones[:], rhs=accA[:, h, :],
                                     start=True, stop=True,
                                     tile_position=(0, h * 64),
                                     skip_group_check=True)
                inv = inv_pool.tile([128, 512], f32, tag="inv",
                                    name=f"inv_{p}_{qc}")
                nc.vector.reciprocal_approx_fast(inv[:], dpsb[:])
                nc.vector.tensor_mul(
                    ATN[:, p, qc * 512:(qc + 1) * 512], avs[:], inv[:])

            # ---------------- main attention blocks ----------------
            # Scores/exp form one continuous high-priority stream; AV steps
            # trail globally (across block boundaries) via a deque so late V
            # blocks or boundary work can never stall the exp stream. Each
            # block's boundary is emitted right after its last AV.
            av_pending = []  # (av_fn, boundary_tuple_or_None)

            def pump_avs(min_keep):
                while len(av_pending) > min_keep:
                    av_fn, bnd = av_pending.pop(0)
                    av_fn()
                    if bnd is not None:
                        emit_boundary(bnd)
                        if bnd[0] == 3 and bnd[1] < NQC - 1:
                            filler_q[3].extend(cproj_fillers(bnd[1]))

            for p in range(NPAIR):
                QTp, KTp = QK[p]
                fq = filler_q[p]
                fi = [0]

                def filler_step(fq=fq, fi=fi):
                    if fi[0] < len(fq):
                        fq[fi[0]]()
                        fi[0] += 1

                for qc in range(NQC):
                    qsl = slice(qc * 512, (qc + 1) * 512)
                    avs = av_pool.tile([128, 512], f32, tag="avs",
                                       name=f"avs_{p}_{qc}")
                    accA = acc_pool.tile([128, 2, 512], bf16, tag="accA",
                                         name=f"accA_{p}_{qc}")
                    pts = {}

                    def scores_step(kb, p=p, qc=qc, QTp=QTp, KTp=KTp,
                                    accA=accA, pts=pts, qsl=qsl):
                        st = st_pool.tile([128, 2, 512], f32, tag="st",
                                          name=f"st_{p}_{qc}_{kb}")
                        ksl = slice(kb * 128, (kb + 1) * 128)
                        pt = pt_pool.tile([128, 2, 512], bf16, tag="pt",
                                          name=f"pt_{p}_{qc}_{kb}")
                        with tc.high_priority(offset=100000):
                            for h in range(2):
                                hsl = slice(64 * h, 64 * h + 64)
                                nc.tensor.matmul(st[:, h, :],
                                                 lhsT=KTp[hsl, ksl],
                                                 rhs=QTp[hsl, qsl],
                                                 start=True, stop=True)
                            nc.scalar.activation(pt[:], st[:], Exp,
                                                 scale=SCALE)
                        pts[kb] = pt
                        if kb == 0:
                            nc.vector.tensor_copy(accA[:], pt[:])
                        else:
                            nc.vector.tensor_add(accA[:], accA[:], pt[:])

                    def av_step(kb, p=p, avs=avs, pts=pts):
                        pt = pts.pop(kb)
                        for h in range(2):
                            nc.tensor.matmul(
                                avs[64 * h:64 * h + 64, :],
                                lhsT=Vt[:, kb, p * 128 + 64 * h:
                                        p * 128 + 64 * h + 64],
                                rhs=pt[:, h, :],
                                start=(kb == 0), stop=(kb == KB - 1),
                                tile_position=(0, 64 * h),
                                skip_group_check=True)

                    cad = cadence[p]
                    for kb in range(KB):
                        scores_step(kb)
                        bnd = (p, qc, avs, accA) if kb == KB - 1 else None
                        av_pending.append(
                            (lambda kb=kb, f=av_step: f(kb), bnd))
                        pump_avs(LAG)
                        if kb % cad == 0:
                            filler_step()
                    filler_step()

                # drain remaining fillers before the next pair needs them
                while fi[0] < len(fq):
                    fq[fi[0]]()
                    fi[0] += 1

            # tail: drain AVs (emits the last boundary) + final c_proj
            lastacc = av_pending[-1][1][3]
            pump_avs(0)
            for q in cproj_fillers(NQC - 1):
                q()
            if dbg_on:
                qtd, ktd = QK[0]
                nc.sync.dma_start(dqt[:, :], qtd[:, :])
                nc.sync.dma_start(dkt[:, :], ktd[:, :])
                nc.sync.dma_start(dvt[:, :, :], Vt[:, :, :])
                nc.sync.dma_start(datn[:, :, :], ATN[:, :, :])
                nc.sync.dma_start(dacc[:, :, :], lastacc[:, :, :])

    nc.compile()
    return nc


def _get_nc():
    if "nc" not in _CACHED:
        _CACHED["nc"] = _build()
    return _CACHED["nc"]


def _shard(x, W_attn, W_proj):
    """Build per-core input maps with pre-swizzled bf16 layouts."""
    bf = ml_dtypes.bfloat16
    x = np.asarray(x, dtype=np.float32)
    W_attn = np.asarray(W_attn, dtype=np.float32)
    W_proj = np.asarray(W_proj, dtype=np.float32)
    in_maps = []
    for c in range(N_CORES):
        b, g = c // 2, c % 2
        # xs[sc, part, kc, j] = x[b, sc*512+j, kc*128+part]
        xt = x[b].T                                  # [D, S]
        xs_ = xt.reshape(KC, 128, NSC, 512).transpose(2, 1, 0, 3)
        # wqk[p, part, t, kc, f] = W_attn[kc*128+part, t*D + g*FC + p*128+f]
        # (partition dim second so the [128,2,KC,128] SBUF tile DMA is a
        # straight linear copy)
        wqk_ = np.empty((NPAIR, 128, 2, KC, 128), dtype=bf)
        for t in range(2):
            wslab = W_attn[:, t * D + g * FC: t * D + (g + 1) * FC]  # [D,FC]
            wr = wslab.reshape(KC, 128, NPAIR, 128).transpose(2, 1, 0, 3)
            wqk_[:, :, t] = wr.astype(bf)
        wv_ = W_attn[:, 2 * D + g * FC: 2 * D + (g + 1) * FC]        # [D,FC]
        wv_ = wv_.reshape(KC, 128, FC).transpose(1, 0, 2).astype(bf)
        # wp[part, p, m] = W_proj[g*FC + p*128 + part, m]
        wp_ = W_proj[g * FC:(g + 1) * FC, :].reshape(NPAIR, 128, D)
        wp_ = wp_.transpose(1, 0, 2).astype(bf)
        in_maps.append({
            "xs": np.ascontiguousarray(xs_.astype(bf)),
            "wqk": np.ascontiguousarray(wqk_),
            "wv": np.ascontiguousarray(wv_),
            "wp": np.ascontiguousarray(wp_),
        })
    return in_maps


def kernel(x, W_attn, W_proj):
    from concourse.bass_utils import run_bass_kernel_spmd

    nc = _get_nc()
    in_maps = _shard(x, W_attn, W_proj)
    trace = os.environ.get("BASS_PROBLEM_TRACE", "0") == "1"
    res = run_bass_kernel_spmd(nc, in_maps, list(range(N_CORES)), trace=trace)
    _CACHED["last_result"] = res
    out = np.empty((B, S, D), dtype=np.float32)
    for b in range(B):
        out[b] = res.results[2 * b]["out"] + res.results[2 * b + 1]["out"]
    return out
